# revision 1
# baseline (speedup 1.0000x reference)
"""KAN layer (histogram binning) Trainium2 kernel.

Math reformulation (exact for linear interpolation on a uniform grid):
  proj = clip(x @ P, +-0.99)                         [N, 3]
  out  = tanh(W @ CPf)  where W[n, (g,c)] = relu(1 - 2.5*|proj[n,c] - grid[g]|)
         and CPf[(g,c), :] = control_points[c, g, :] * component_weights[c]

Kernel pipeline per 512-token quarter (8 cores x 8192 tokens each):
  DMA x in (natural [tok, d]) -> PE transpose to [d, tok] -> U = Q^T.T @ X^T
  (one fused matmul, Q = P replicated over grid rows) -> clip (DVE) ->
  |u - g| (DVE fused add+abs_max with per-partition bias) -> relu affine (ACT)
  -> out = W.T @ CPf per 128-token chunk (back to [tok, o] layout) -> tanh
  (ACT) -> DMA out.
"""

import os
from contextlib import ExitStack

import numpy as np

import concourse.bass as bass
import concourse.bacc as bacc
import concourse.tile as tile
from concourse import mybir
from concourse.bass_utils import run_bass_kernel_spmd

N_CORES = 8
TOK_TOTAL = 32 * 2048
D = 256
O = 256
G = 6
C = 3
R = G * C  # 18 interp weights per token
SUPER = 2048  # tokens per supertile
QUART = 512
CHUNK = 128

F32 = mybir.dt.float32
F32R = mybir.dt.float32r

# dtype knobs (bitcast f32 -> f32r at the instruction site)
MMU_F32R = os.environ.get("KAN_MMU_F32R", "0") == "1"
MM3_F32R = os.environ.get("KAN_MM3_F32R", "1") == "1"
TRANS_F32R = os.environ.get("KAN_TRANS_F32R", "0") == "1"


MMU_DT = F32R if MMU_F32R else F32
MM3_DT = F32R if MM3_F32R else F32
TRANS_DT = F32R if TRANS_F32R else F32


def build_nc(tok_per_core: int, n_cores: int = N_CORES):
    n_super = tok_per_core // SUPER
    assert tok_per_core % SUPER == 0

    nc = bacc.Bacc(
        "TRN2", target_bir_lowering=False, debug=False, num_devices=n_cores
    )
    x_d = nc.dram_tensor("x", [tok_per_core, D], F32, kind="ExternalInput").ap()
    q_d = nc.dram_tensor("qmat", [128, 2 * R], MMU_DT, kind="ExternalInput").ap()
    cp_d = nc.dram_tensor("cpb", [32, O], MM3_DT, kind="ExternalInput").ap()
    b_d = nc.dram_tensor("biasp", [32, 1], F32, kind="ExternalInput").ap()
    id_d = nc.dram_tensor("ident", [128, 128], TRANS_DT, kind="ExternalInput").ap()
    out_d = nc.dram_tensor("out", [tok_per_core, D], F32, kind="ExternalOutput").ap()

    x_v = x_d.rearrange("(s j p) d -> s p j d", p=128, j=SUPER // CHUNK)
    o_v = out_d.rearrange("(s j p) d -> s p j d", p=128, j=SUPER // CHUNK)

    with tile.TileContext(nc) as tc, ExitStack() as ctx:
        const_p = ctx.enter_context(tc.tile_pool(name="const", bufs=1))
        xn_p = ctx.enter_context(tc.tile_pool(name="xn", bufs=2))
        xtps_p = ctx.enter_context(tc.tile_pool(name="xtps", bufs=3, space="PSUM"))
        xtsb_p = ctx.enter_context(tc.tile_pool(name="xtsb", bufs=3))
        ups_p = ctx.enter_context(tc.tile_pool(name="ups", bufs=2, space="PSUM"))
        w_p = ctx.enter_context(tc.tile_pool(name="w", bufs=2))
        ops_p = ctx.enter_context(tc.tile_pool(name="ops", bufs=3, space="PSUM"))
        osb_p = ctx.enter_context(tc.tile_pool(name="osb", bufs=2))

        ident = const_p.tile([128, 128], TRANS_DT)
        nc.sync.dma_start(ident[:], id_d)
        qmat = const_p.tile([128, 2 * R], MMU_DT)
        nc.sync.dma_start(qmat[:], q_d)
        cpb = const_p.tile([32, O], MM3_DT)
        nc.sync.dma_start(cpb[:], cp_d)
        biasp = const_p.tile([32, 1], F32)
        nc.sync.dma_start(biasp[:], b_d)

        for s in range(n_super):
            xn = xn_p.tile([128, SUPER // CHUNK * D], TRANS_DT)
            nc.sync.dma_start(
                xn.rearrange("p (j d) -> p j d", j=SUPER // CHUNK), x_v[s]
            )
            out_sb = osb_p.tile([128, SUPER // CHUNK * O], F32)
            for q in range(SUPER // QUART):
                xt_sb = []
                for h in range(2):
                    xt_ps = xtps_p.tile([128, QUART], TRANS_DT, tag="xtps")
                    for c in range(QUART // CHUNK):
                        j = (SUPER // QUART) * q + c
                        nc.tensor.transpose(
                            xt_ps[:, CHUNK * c : CHUNK * (c + 1)],
                            xn[:, j * D + 128 * h : j * D + 128 * (h + 1)],
                            ident[:],
                        )
                    sb = xtsb_p.tile([128, QUART], MMU_DT, tag="xtsb")
                    nc.vector.tensor_copy(sb[:], xt_ps[:])
                    xt_sb.append(sb)
                u_ps = ups_p.tile([128, QUART], F32, tag="ups")
                for h in range(2):
                    nc.tensor.matmul(
                        u_ps[0:R, :],
                        qmat[:, R * h : R * (h + 1)],
                        xt_sb[h][:],
                        start=(h == 0),
                        stop=(h == 1),
                    )
                u_sb = w_p.tile([32, QUART], F32, tag="usb")
                nc.vector.tensor_scalar(
                    u_sb[0:R, :],
                    u_ps[0:R, :],
                    0.99,
                    -0.99,
                    op0=mybir.AluOpType.min,
                    op1=mybir.AluOpType.max,
                )
                a_sb = w_p.tile([32, QUART], F32, tag="asb")
                nc.scalar.activation(
                    a_sb[0:R, :],
                    u_sb[0:R, :],
                    mybir.ActivationFunctionType.Abs,
                    bias=biasp[0:R, :],
                    scale=1.0,
                )
                w_sb = w_p.tile([32, QUART], MM3_DT, tag="wsb")
                nc.scalar.activation(
                    w_sb[0:R, :],
                    a_sb[0:R, :],
                    mybir.ActivationFunctionType.Relu,
                    bias=1.0,
                    scale=-2.5,
                )
                for cp_i in range(2):
                    out_ps = ops_p.tile([128, 2 * O], F32, tag="ops")
                    for ce in range(2):
                        c = 2 * cp_i + ce
                        nc.tensor.matmul(
                            out_ps[:, O * ce : O * (ce + 1)],
                            w_sb[0:R, CHUNK * c : CHUNK * (c + 1)],
                            cpb[0:R, :],
                            start=True,
                            stop=True,
                        )
                    off = ((SUPER // QUART) * q + 2 * cp_i) * O
                    nc.scalar.activation(
                        out_sb[:, off : off + 2 * O],
                        out_ps[:],
                        mybir.ActivationFunctionType.Tanh,
                    )
            nc.sync.dma_start(
                o_v[s], out_sb.rearrange("p (j d) -> p j d", j=SUPER // CHUNK)
            )

    nc.compile()
    return nc


def make_consts(projections: np.ndarray, control_points: np.ndarray,
                component_weights: np.ndarray):
    grid = np.linspace(-1.0, 1.0, G).astype(np.float32)
    qmat = np.zeros((128, 2 * R), np.float32)
    for h in range(2):
        for r in range(R):
            qmat[:, h * R + r] = projections[h * 128 : (h + 1) * 128, r % C]
    cpb = np.zeros((32, O), np.float32)
    biasp = np.zeros((32, 1), np.float32)
    for r in range(R):
        g, c = r // C, r % C
        cpb[r] = control_points[c, g] * component_weights[c]
        biasp[r, 0] = -grid[g]
    ident = np.eye(128, dtype=np.float32)
    return qmat, cpb, biasp, ident


_NC_CACHE = {}


def kernel(x, projections, control_points, component_weights, _trace=False):
    x = np.asarray(x)
    B, S, _ = x.shape
    tok = B * S
    tok_per_core = tok // N_CORES
    key = tok_per_core
    if key not in _NC_CACHE:
        _NC_CACHE[key] = build_nc(tok_per_core)
    nc = _NC_CACHE[key]

    qmat, cpb, biasp, ident = make_consts(
        np.asarray(projections), np.asarray(control_points),
        np.asarray(component_weights)
    )
    flat = np.ascontiguousarray(x.reshape(tok, D))
    in_maps = []
    for i in range(N_CORES):
        in_maps.append(
            {
                "x": flat[i * tok_per_core : (i + 1) * tok_per_core],
                "qmat": qmat,
                "cpb": cpb,
                "biasp": biasp,
                "ident": ident,
            }
        )
    res = run_bass_kernel_spmd(nc, in_maps, list(range(N_CORES)), trace=_trace)
    out = np.concatenate([res.results[i]["out"] for i in range(N_CORES)], axis=0)
    ret = out.reshape(B, S, O).astype(x.dtype, copy=False)
    if _trace:
        return ret, res
    return ret



# revision 7
# speedup vs baseline: 1720.0333x; 1720.0333x over previous
"""KAN layer (histogram binning) Trainium2 kernel — transposeless bf16 design.

Math reformulation (exact for linear interpolation on a uniform grid, hat
basis):
  proj = clip(x @ P, +-0.99)                          [N, 3]
  out  = tanh(sum_r w'[n, r] * cpbn[r, :])
  where, per (grid g, component c) row r = g*C + c:
    w'[n, r]   = min(|proj[n, c] - grid[g]| - 0.4, 0)   (= -relu(0.4 - |d|))
    cpbn[r, :] = -2.5 * control_points[c, g, :] * component_weights[c]
  (relu(1 - 2.5|d|) = 2.5 * relu(0.4 - |d|); both minus signs cancel in the
  matmul, so no extra negate instruction is needed.)

Layout: the host uploads x pre-transposed per core ([D, tok] fp16), so the
kernel needs NO PE transpose and no PSUM->SBUF staging copy for x. Per
512-token quarter:
  u = qmat^T @ xT (2 accumulating bf16 matmuls over the two 128-feature
  halves) -> clip (DVE min/max) -> |u - g| (DVE add + abs_max, per-partition
  bias) -> w' (DVE subtract/min, bf16 out) -> out = w'^T @ cpbn per 128-token
  chunk -> tanh (ACT, [128, 1024] per instruction, bf16 out) -> DMA out
  (p-major dram layout: one contiguous 8 KiB segment per partition per
  supertile).

Host side: shard tokens across 8 cores, upload xT bf16, download bf16
p-major output, reorder + upcast to f32.
"""

from contextlib import ExitStack, nullcontext

import numpy as np

import concourse.bass as bass
import concourse.bacc as bacc
import concourse.tile as tile
from concourse import mybir
from concourse.bass_utils import run_bass_kernel_spmd

N_CORES = 8
TOK_TOTAL = 32 * 2048
D = 256
O = 256
G = 6
C = 3
R = G * C  # 18 hat-basis rows
SUPER = 2048  # tokens per supertile
QUART = 512  # tokens per PSUM-bank-sized quarter
CHUNK = 128  # tokens per output matmul (partition dim)

F32 = mybir.dt.float32
F32R = mybir.dt.float32r
F16 = mybir.dt.float16
BF16 = mybir.dt.bfloat16
BF16_NP = mybir.dt.np(mybir.dt.bfloat16)


def build_nc(tok_per_core: int, repeat: int = 1, n_cores: int = N_CORES):
    """Build the per-core kernel. `repeat` wraps the whole body in a hardware
    For_i loop (used only by benchmarking to amortize dispatch overhead)."""
    n_super = tok_per_core // SUPER
    assert tok_per_core % SUPER == 0

    nc = bacc.Bacc(
        "TRN2", target_bir_lowering=False, debug=False, num_devices=n_cores
    )
    xt_d = nc.dram_tensor("xt", [D, tok_per_core], F16, kind="ExternalInput").ap()
    q_d = nc.dram_tensor("qmat", [128, 2 * R], F16, kind="ExternalInput").ap()
    cp_d = nc.dram_tensor("cpbn", [32, O], BF16, kind="ExternalInput").ap()
    b_d = nc.dram_tensor("biasp", [32, 1], F32, kind="ExternalInput").ap()
    out_d = nc.dram_tensor(
        "out", [n_super * 128, (SUPER // CHUNK) * O], BF16, kind="ExternalOutput"
    ).ap()

    # xt[(h*128+p), (s*SUPER+t)] -> [s, p, h, t]: per partition, two
    # contiguous 2*SUPER-byte segments per supertile.
    x_v = xt_d.rearrange("(h p) (s t) -> s p h t", p=128, t=SUPER)
    # out[(s*128+p), (j*O+o)]: one contiguous 8 KiB segment per partition
    # per supertile.
    o_v = out_d.rearrange("(s p) f -> s p f", p=128)

    with tile.TileContext(nc) as tc, ExitStack() as ctx:
        const_p = ctx.enter_context(tc.tile_pool(name="const", bufs=1))
        xn_p = ctx.enter_context(tc.tile_pool(name="xn", bufs=2))
        ups_p = ctx.enter_context(tc.tile_pool(name="ups", bufs=2, space="PSUM"))
        w_p = ctx.enter_context(tc.tile_pool(name="w", bufs=2))
        ops_p = ctx.enter_context(tc.tile_pool(name="ops", bufs=2, space="PSUM"))
        osb_p = ctx.enter_context(tc.tile_pool(name="osb", bufs=2))

        qmat = const_p.tile([128, 2 * R], F16)
        nc.sync.dma_start(qmat[:], q_d)
        cpbn = const_p.tile([32, O], BF16)
        nc.sync.dma_start(cpbn[:], cp_d)
        biasp = const_p.tile([32, 1], F32)
        nc.sync.dma_start(biasp[:], b_d)

        loop_cm = tc.For_i(0, repeat, 1) if repeat > 1 else nullcontext()
        with loop_cm:
            for s in range(n_super):
                xt = xn_p.tile([128, 2 * SUPER], F16)
                nc.sync.dma_start(
                    xt.rearrange("p (h t) -> p h t", h=2), x_v[s]
                )
                out_sb = osb_p.tile([128, (SUPER // CHUNK) * O], BF16)
                for q in range(SUPER // QUART):
                    u_ps = ups_p.tile([128, QUART], F32, tag="ups")
                    nc.tensor.matmul(
                        u_ps[0:R, :],
                        qmat[:, 0:R],
                        xt[:, QUART * q : QUART * (q + 1)],
                        start=True,
                        stop=False,
                    )
                    nc.tensor.matmul(
                        u_ps[0:R, :],
                        qmat[:, R : 2 * R],
                        xt[:, SUPER + QUART * q : SUPER + QUART * (q + 1)],
                        start=False,
                        stop=True,
                    )
                    u_sb = w_p.tile([32, QUART], F32, tag="usb")
                    nc.vector.tensor_scalar(
                        u_sb[0:R, :],
                        u_ps[0:R, :],
                        0.99,
                        -0.99,
                        op0=mybir.AluOpType.min,
                        op1=mybir.AluOpType.max,
                    )
                    a_sb = w_p.tile([32, QUART], F32, tag="asb")
                    nc.scalar.activation(
                        a_sb[0:R, :],
                        u_sb[0:R, :],
                        mybir.ActivationFunctionType.Abs,
                        bias=biasp[0:R, :],
                        scale=1.0,
                    )
                    w_sb = w_p.tile([32, QUART], BF16, tag="wsb")
                    nc.vector.tensor_scalar(
                        w_sb[0:R, :],
                        a_sb[0:R, :],
                        0.4,
                        0.0,
                        op0=mybir.AluOpType.subtract,
                        op1=mybir.AluOpType.min,
                    )
                    out_ps = ops_p.tile([128, 4 * O], F32, tag="ops")
                    for k in range(QUART // CHUNK):
                        nc.tensor.matmul(
                            out_ps[:, O * k : O * (k + 1)],
                            w_sb[0:R, CHUNK * k : CHUNK * (k + 1)],
                            cpbn[0:R, :],
                            start=True,
                            stop=True,
                        )
                    nc.scalar.activation(
                        out_sb[:, 4 * O * q : 4 * O * (q + 1)],
                        out_ps[:],
                        mybir.ActivationFunctionType.Tanh,
                    )
                nc.sync.dma_start(o_v[s], out_sb[:])

    nc.compile()
    return nc


def make_consts(projections: np.ndarray, control_points: np.ndarray,
                component_weights: np.ndarray):
    grid = np.linspace(-1.0, 1.0, G).astype(np.float32)
    qmat = np.zeros((128, 2 * R), np.float32)
    for h in range(2):
        for r in range(R):
            qmat[:, h * R + r] = projections[h * 128 : (h + 1) * 128, r % C]
    cpbn = np.zeros((32, O), np.float32)
    biasp = np.zeros((32, 1), np.float32)
    for r in range(R):
        g, c = r // C, r % C
        cpbn[r] = -2.5 * control_points[c, g] * component_weights[c]
        biasp[r, 0] = -grid[g]
    return qmat.astype(np.float16), cpbn.astype(BF16_NP), biasp


def make_in_maps(x: np.ndarray, projections: np.ndarray,
                 control_points: np.ndarray, component_weights: np.ndarray):
    tok = x.shape[0] * x.shape[1]
    tok_per_core = tok // N_CORES
    qmat, cpbn, biasp = make_consts(
        np.asarray(projections), np.asarray(control_points),
        np.asarray(component_weights)
    )
    flat = x.reshape(tok, D)
    in_maps = []
    for i in range(N_CORES):
        xt = flat[i * tok_per_core : (i + 1) * tok_per_core].T.astype(np.float16)
        in_maps.append(
            {"xt": np.ascontiguousarray(xt), "qmat": qmat, "cpbn": cpbn,
             "biasp": biasp}
        )
    return in_maps


def assemble_output(outs, B, S):
    """outs: per-core [n_super*128, 16*O] bf16 p-major arrays -> [B, S, O] f32."""
    tok_per_core = TOK_TOTAL // N_CORES
    n_super = tok_per_core // SUPER
    parts = []
    for o in outs:
        a = np.asarray(o).reshape(n_super, 128, SUPER // CHUNK, O)
        parts.append(
            a.transpose(0, 2, 1, 3).reshape(tok_per_core, O).astype(np.float32)
        )
    return np.concatenate(parts, axis=0).reshape(B, S, O)


_NC_CACHE = {}


def kernel(x, projections, control_points, component_weights, _trace=False):
    x = np.asarray(x)
    B, S, _ = x.shape
    tok = B * S
    tok_per_core = tok // N_CORES
    key = (tok_per_core, 1)
    if key not in _NC_CACHE:
        _NC_CACHE[key] = build_nc(tok_per_core)
    nc = _NC_CACHE[key]

    in_maps = make_in_maps(x, projections, control_points, component_weights)
    res = run_bass_kernel_spmd(nc, in_maps, list(range(N_CORES)), trace=_trace)
    ret = assemble_output(
        [res.results[i]["out"] for i in range(N_CORES)], B, S
    ).astype(x.dtype, copy=False)
    if _trace:
        return ret, res
    return ret
